# revision 9
# baseline (speedup 1.0000x reference)
"""GCNConv Trainium2 kernel: out = D^{-1/2} A D^{-1/2} (X @ W).

Strategy (8 NeuronCores, 1D row partition of the uniform-degree CSR):
  - each core owns 12500 destination nodes (padded to 12544 = 98*128)
  - phase A: X_k @ W in bf16 (X pre-transposed on host, so no PE
    transposes), row-scaled by d_j -> bf16 X'' shard
  - phase B: AllGather bf16 shards -> full [100352, 64] bf16 table
  - phase C: per 128-node tile, two dma_gather calls fetch the 16
    neighbor rows of each node from the 4-packed bf16 table (512B
    rows, idx = node//4 fits int16); DVE residue-mask select + reduce,
    scale by d_i, store f32.
Host side: shard/pad inputs, transpose X, remap column indices into
the packed AllGather layout, unshard the output.
"""

import numpy as np
import ml_dtypes

N_NODES = 100000
D_IN = 256
D_OUT = 64
DEG = 16
N_CORES = 8
P = 128
SHARD = N_NODES // N_CORES            # 12500
N_TILES = (SHARD + P - 1) // P        # 98
NPAD = N_TILES * P                    # 12544

_CACHE = {}


def _build_program(n_tiles=N_TILES, deg=DEG, d_in=D_IN, d_out=D_OUT,
                   n_cores=N_CORES, debug_taps=False):
    import concourse.bacc as bacc
    from concourse import bass, mybir, tile

    npad = n_tiles * P
    f32 = mybir.dt.float32
    bf16 = mybir.dt.bfloat16
    i16 = mybir.dt.int16

    nc = bacc.Bacc("TRN2", target_bir_lowering=False, debug=False,
                   num_devices=n_cores)
    XsT = nc.dram_tensor("XsT", [d_in, npad], bf16, kind="ExternalInput").ap()
    W = nc.dram_tensor("W", [d_in, d_out], bf16, kind="ExternalInput").ap()
    degs = nc.dram_tensor("degs", [npad, 1], f32, kind="ExternalInput").ap()
    # packed-gather inputs: node//4 indices (int16, two 1024-idx calls per
    # tile) + residue one-hot masks
    idxs = nc.dram_tensor("idxs", [n_tiles, 2, P, (deg // 2) * P // 16],
                          i16, kind="ExternalInput").ap()
    msks = nc.dram_tensor("msks", [n_tiles, P, 4 * deg], bf16,
                          kind="ExternalInput").ap()
    out = nc.dram_tensor("out", [npad, d_out], f32, kind="ExternalOutput").ap()
    if debug_taps:
        xpd_out = nc.dram_tensor("xpd_out", [npad, d_out], bf16,
                                 kind="ExternalOutput").ap()
        xfull_out = nc.dram_tensor("xfull_out", [n_cores * npad, d_out], bf16,
                                   kind="ExternalOutput").ap()

    n_kchunk = d_in // P  # 2
    half = deg // 2
    pk = 4 * d_out  # 256 bf16 elems = 512B per packed row

    with tile.TileContext(nc) as tc:
        with (
            tc.tile_pool(name="const", bufs=1) as constp,
            tc.tile_pool(name="xin", bufs=3) as xinp,
            tc.tile_pool(name="ps", bufs=4, space="PSUM") as psp,
            tc.tile_pool(name="xp", bufs=3) as xpp,
            tc.tile_pool(name="dg", bufs=3) as degp,
            tc.tile_pool(name="ix", bufs=8) as idxp,
            tc.tile_pool(name="gt", bufs=3) as gp,
            tc.tile_pool(name="ot", bufs=3) as outp,
            tc.tile_pool(name="dram", bufs=1, space="DRAM") as dramp,
        ):
            w_sb = constp.tile([P, n_kchunk * d_out], bf16)
            for c in range(n_kchunk):
                nc.sync.dma_start(out=w_sb[:, c * d_out:(c + 1) * d_out],
                                  in_=W[c * P:(c + 1) * P, :])

            xpd = dramp.tile([npad, d_out], bf16)
            xfull = dramp.tile([n_cores * npad, d_out], bf16,
                               addr_space="Shared")

            # ---- Phase A: X'' = (X @ W) * d_j  (bf16) ----
            for t in range(n_tiles):
                sl = slice(t * P, (t + 1) * P)
                xT = xinp.tile([P, n_kchunk, P], bf16)
                for c in range(n_kchunk):
                    nc.sync.dma_start(out=xT[:, c, :],
                                      in_=XsT[c * P:(c + 1) * P, sl])
                deg_t = degp.tile([P, 1], f32)
                nc.sync.dma_start(out=deg_t[:], in_=degs[sl, :])
                pso = psp.tile([P, d_out], f32, space="PSUM")
                for c in range(n_kchunk):
                    nc.tensor.matmul(pso[:], xT[:, c, :],
                                     w_sb[:, c * d_out:(c + 1) * d_out],
                                     start=(c == 0), stop=(c == n_kchunk - 1))
                xp_t = xpp.tile([P, d_out], bf16)
                nc.vector.tensor_scalar_mul(xp_t[:], pso[:], deg_t[:, 0:1])
                nc.sync.dma_start(out=xpd[sl, :], in_=xp_t[:])

            # ---- Phase B: AllGather shards ----
            nc.gpsimd.collective_compute(
                "AllGather", mybir.AluOpType.bypass,
                replica_groups=[list(range(n_cores))],
                ins=[xpd.opt()], outs=[xfull.opt()],
            )

            if debug_taps:
                nc.sync.dma_start(out=xpd_out[:], in_=xpd[:])
                nc.sync.dma_start(out=xfull_out[:], in_=xfull[:])

            # ---- Phase C: bulk dma_gather on the 4-packed bf16 table
            # (idx = node//4 fits int16), residue-mask select + reduce ----
            xpk = xfull[:].rearrange("(a b) f -> a (b f)", b=4)  # [N/4, 256]
            for t in range(n_tiles):
                sl = slice(t * P, (t + 1) * P)
                msk_t = idxp.tile([P, 4 * deg], bf16, tag="msk")
                nc.sync.dma_start(out=msk_t[:], in_=msks[t])
                deg_c = degp.tile([P, 1], f32, tag="deg_c")
                nc.sync.dma_start(out=deg_c[:], in_=degs[sl, :])
                rs = []
                for h in range(2):
                    idx_t = idxp.tile([P, half * P // 16], i16, tag="idx")
                    nc.sync.dma_start(out=idx_t[:], in_=idxs[t, h])
                    g_h = gp.tile([P, half * pk], bf16, tag=f"g{h}")
                    nc.gpsimd.dma_gather(
                        g_h[:].rearrange("p (s f) -> p s f", s=half),
                        xpk, idx_t[:], half * P, half * P, pk)
                    prod = gp.tile([P, half * pk], bf16, tag=f"prod{h}")
                    nc.vector.tensor_tensor(
                        out=prod[:].rearrange("p (s q f) -> p s q f",
                                              s=half, q=4),
                        in0=g_h[:].rearrange("p (s q f) -> p s q f",
                                             s=half, q=4),
                        in1=msk_t[:, h * 4 * half:(h + 1) * 4 * half]
                        .rearrange("p (s q) -> p s q", q=4)
                        .to_broadcast([P, half, 4, d_out]),
                        op=mybir.AluOpType.mult)
                    r_h = outp.tile([P, d_out], f32, tag=f"r{h}")
                    nc.vector.tensor_reduce(
                        r_h[:],
                        prod[:].rearrange("p (s q f) -> p f s q",
                                          s=half, q=4),
                        axis=mybir.AxisListType.XY, op=mybir.AluOpType.add)
                    rs.append(r_h)
                nc.vector.tensor_add(rs[0][:], rs[0][:], rs[1][:])
                o_t = outp.tile([P, d_out], f32, tag="o_t")
                nc.vector.tensor_scalar_mul(o_t[:], rs[0][:], deg_c[:, 0:1])
                nc.sync.dma_start(out=out[sl, :], in_=o_t[:])

    nc.compile()
    return nc


def _get_program():
    key = "main"
    if key not in _CACHE:
        _CACHE[key] = _build_program()
    return _CACHE[key]


def _prep_inputs(X, weights, column_index, degrees,
                 n_nodes=N_NODES, n_cores=N_CORES, shard=SHARD, npad=NPAD,
                 deg=DEG):
    """Shard + pad host arrays; remap columns to packed AllGather layout."""
    X = np.asarray(X, dtype=np.float32)
    W = (np.asarray(weights, dtype=np.float32)
         .astype(ml_dtypes.bfloat16))
    col = np.asarray(column_index).astype(np.int64, copy=False)
    dg = np.asarray(degrees, dtype=np.float32)

    # remap node id -> row in the AllGather-concatenated padded table
    col32 = (col // shard * npad + col % shard).astype(np.int32)
    col32 = col32.reshape(n_cores, shard, deg)

    n_tiles = npad // P
    half = deg // 2
    in_maps = []
    pad = npad - shard
    iw = np.arange(half * P)
    for c in range(n_cores):
        XcT = np.zeros((X.shape[1], npad), np.float32)
        XcT[:, :shard] = X[c * shard:(c + 1) * shard].T
        dgc = np.concatenate(
            [dg[c * shard:(c + 1) * shard],
             np.zeros(pad, np.float32)], axis=0).reshape(npad, 1)
        ixc = np.concatenate(
            [col32[c], np.zeros((pad, deg), np.int32)], axis=0)
        q4, r4 = ixc // 4, ixc % 4                      # [npad, deg]
        # idx16[t, h]: wrapped int16 layout for 1024-idx dma_gather calls;
        # logical i = s_local*128 + p -> wrap[i%16, i//16], tiled to 128 rows
        idx16 = np.zeros((n_tiles, 2, P, half * P // 16), np.int16)
        for t in range(n_tiles):
            blk = q4[t * P:(t + 1) * P]                 # [128, deg]
            for h in range(2):
                arr = blk[:, h * half:(h + 1) * half].T.reshape(-1)
                wrap = np.zeros((16, half * P // 16), np.int16)
                wrap[iw % 16, iw // 16] = arr
                idx16[t, h] = np.tile(wrap, (8, 1))
        # msk[t, p, s*4+q] = 1.0 where residue matches
        msk = (r4[:, :, None] == np.arange(4)[None, None, :]).astype(
            np.float32).reshape(n_tiles, P, deg * 4).astype(ml_dtypes.bfloat16)
        in_maps.append({"XsT": XcT.astype(ml_dtypes.bfloat16), "W": W,
                        "degs": dgc, "idxs": idx16, "msks": msk})
    return in_maps


def kernel(X, weights, row_pointers, column_index, degrees):
    from concourse.bass_utils import run_bass_kernel_spmd

    rp = np.asarray(row_pointers)
    assert rp.shape[0] == N_NODES + 1
    in_maps = _prep_inputs(X, weights, column_index, degrees)
    nc = _get_program()
    res = run_bass_kernel_spmd(nc, in_maps, core_ids=list(range(N_CORES)))
    outs = [res.results[c]["out"][:SHARD] for c in range(N_CORES)]
    return np.concatenate(outs, axis=0)


# revision 11
# speedup vs baseline: 2.5232x; 2.5232x over previous
"""GCNConv Trainium2 kernel: out = D^{-1/2} A D^{-1/2} (X @ W).

Strategy (8 NeuronCores, 1D row partition of the uniform-degree CSR):
  - each core owns 12500 destination nodes (padded to 12544 = 98*128)
  - phase A: (d_j X)_k @ W in bf16 (X pre-scaled by d_j and
    pre-transposed on host, so no PE transposes and no post-scale;
    PSUM -> bf16 via the scalar engine)
  - phase B: AllGather bf16 shards -> full [100352, 64] bf16 table
  - phase C: per 128-node tile, two dma_gather calls (round-robin over
    4 SWDGE queues) fetch the 16 neighbor rows of each node from the
    4-packed bf16 table (512B rows, idx = node//4 fits int16); DVE
    applies d_i-premultiplied residue masks then a contiguous
    halving-add tree, store f32.
Host side: shard/pad inputs, fold both degree scalings, transpose X,
remap column indices into the packed AllGather layout.
"""

import numpy as np
import ml_dtypes

N_NODES = 100000
D_IN = 256
D_OUT = 64
DEG = 16
N_CORES = 8
P = 128
SHARD = N_NODES // N_CORES            # 12500
N_TILES = (SHARD + P - 1) // P        # 98
NPAD = N_TILES * P                    # 12544

_CACHE = {}


def _build_program(n_tiles=N_TILES, deg=DEG, d_in=D_IN, d_out=D_OUT,
                   n_cores=N_CORES, debug_taps=False):
    import concourse.bacc as bacc
    from concourse import bass, mybir, tile

    npad = n_tiles * P
    f32 = mybir.dt.float32
    bf16 = mybir.dt.bfloat16
    i16 = mybir.dt.int16

    nc = bacc.Bacc("TRN2", target_bir_lowering=False, debug=False,
                   num_devices=n_cores, num_swdge_queues=4)
    XsT = nc.dram_tensor("XsT", [d_in, npad], bf16, kind="ExternalInput").ap()
    W = nc.dram_tensor("W", [d_in, d_out], bf16, kind="ExternalInput").ap()
    # packed-gather inputs: node//4 indices (int16, two 1024-idx calls per
    # tile) + residue one-hot masks premultiplied by d_i
    idxs = nc.dram_tensor("idxs", [n_tiles, 2, P, (deg // 2) * P // 16],
                          i16, kind="ExternalInput").ap()
    msks = nc.dram_tensor("msks", [n_tiles, P, 4 * deg], bf16,
                          kind="ExternalInput").ap()
    out = nc.dram_tensor("out", [npad, d_out], f32, kind="ExternalOutput").ap()
    if debug_taps:
        xpd_out = nc.dram_tensor("xpd_out", [npad, d_out], bf16,
                                 kind="ExternalOutput").ap()
        xfull_out = nc.dram_tensor("xfull_out", [n_cores * npad, d_out], bf16,
                                   kind="ExternalOutput").ap()

    n_kchunk = d_in // P  # 2
    half = deg // 2
    pk = 4 * d_out  # 256 bf16 elems = 512B per packed row

    with tile.TileContext(nc) as tc:
        with (
            tc.tile_pool(name="const", bufs=1) as constp,
            tc.tile_pool(name="xin", bufs=3) as xinp,
            tc.tile_pool(name="ps", bufs=4, space="PSUM") as psp,
            tc.tile_pool(name="xp", bufs=3) as xpp,
            tc.tile_pool(name="ix", bufs=8) as idxp,
            tc.tile_pool(name="gt", bufs=4) as gp,
            tc.tile_pool(name="rd", bufs=3) as rdp,
            tc.tile_pool(name="ot", bufs=3) as outp,
            tc.tile_pool(name="dram", bufs=1, space="DRAM") as dramp,
        ):
            w_sb = constp.tile([P, n_kchunk * d_out], bf16)
            for c in range(n_kchunk):
                nc.sync.dma_start(out=w_sb[:, c * d_out:(c + 1) * d_out],
                                  in_=W[c * P:(c + 1) * P, :])

            xpd = dramp.tile([npad, d_out], bf16)
            xfull = dramp.tile([n_cores * npad, d_out], bf16,
                               addr_space="Shared")

            # ---- Phase A: X'' = (d_j X) @ W  (bf16) ----
            for t in range(n_tiles):
                sl = slice(t * P, (t + 1) * P)
                xT = xinp.tile([P, n_kchunk, P], bf16)
                for c in range(n_kchunk):
                    nc.sync.dma_start(out=xT[:, c, :],
                                      in_=XsT[c * P:(c + 1) * P, sl])
                pso = psp.tile([P, d_out], f32, space="PSUM")
                for c in range(n_kchunk):
                    nc.tensor.matmul(pso[:], xT[:, c, :],
                                     w_sb[:, c * d_out:(c + 1) * d_out],
                                     start=(c == 0), stop=(c == n_kchunk - 1))
                xp_t = xpp.tile([P, d_out], bf16)
                nc.scalar.copy(xp_t[:], pso[:])
                nc.sync.dma_start(out=xpd[sl, :], in_=xp_t[:])

            # ---- Phase B: AllGather shards ----
            nc.gpsimd.collective_compute(
                "AllGather", mybir.AluOpType.bypass,
                replica_groups=[list(range(n_cores))],
                ins=[xpd.opt()], outs=[xfull.opt()],
            )

            if debug_taps:
                nc.sync.dma_start(out=xpd_out[:], in_=xpd[:])
                nc.sync.dma_start(out=xfull_out[:], in_=xfull[:])

            # ---- Phase C: bulk dma_gather on the 4-packed bf16 table,
            # mask-mult (d_i folded in), contiguous halving-add tree ----
            xpk = xfull[:].rearrange("(a b) f -> a (b f)", b=4)  # [N/4, 256]
            for t in range(n_tiles):
                sl = slice(t * P, (t + 1) * P)
                msk_t = idxp.tile([P, 4 * deg], bf16, tag="msk")
                nc.sync.dma_start(out=msk_t[:], in_=msks[t])
                prods = []
                for h in range(2):
                    idx_t = idxp.tile([P, half * P // 16], i16, tag="idx")
                    nc.sync.dma_start(out=idx_t[:], in_=idxs[t, h])
                    g_h = gp.tile([P, half * pk], bf16, tag=f"g{h}")
                    nc.gpsimd.dma_gather(
                        g_h[:].rearrange("p (s f) -> p s f", s=half),
                        xpk, idx_t[:], half * P, half * P, pk,
                        queue_num=(2 * t + h) % 4)
                    prod = gp.tile([P, half * pk], bf16, tag=f"prod{h}")
                    nc.vector.tensor_tensor(
                        out=prod[:].rearrange("p (s q f) -> p s q f",
                                              s=half, q=4),
                        in0=g_h[:].rearrange("p (s q f) -> p s q f",
                                             s=half, q=4),
                        in1=msk_t[:, h * 4 * half:(h + 1) * 4 * half]
                        .rearrange("p (s q) -> p s q", q=4)
                        .to_broadcast([P, half, 4, d_out]),
                        op=mybir.AluOpType.mult)
                    prods.append(prod)
                # halving-add tree over (s=half, q=4) -> contiguous slices
                width = half * 4 * d_out
                a0 = rdp.tile([P, width], bf16, tag="a0")
                nc.vector.tensor_add(a0[:], prods[0][:], prods[1][:])
                cur, step = a0, 0
                while width > 2 * d_out:
                    width //= 2
                    nxt = rdp.tile([P, width], bf16, tag=f"h{step}")
                    nc.vector.tensor_add(nxt[:], cur[:, :width],
                                         cur[:, width:2 * width])
                    cur, step = nxt, step + 1
                o_t = outp.tile([P, d_out], f32, tag="o_t")
                nc.vector.tensor_add(o_t[:], cur[:, :d_out],
                                     cur[:, d_out:2 * d_out])
                nc.sync.dma_start(out=out[sl, :], in_=o_t[:])

    nc.compile()
    return nc


def _get_program():
    key = "main"
    if key not in _CACHE:
        _CACHE[key] = _build_program()
    return _CACHE[key]


def _prep_inputs(X, weights, column_index, degrees,
                 n_nodes=N_NODES, n_cores=N_CORES, shard=SHARD, npad=NPAD,
                 deg=DEG):
    """Shard + pad host arrays; fold degree scalings; remap columns to the
    packed AllGather layout."""
    X = np.asarray(X, dtype=np.float32)
    W = (np.asarray(weights, dtype=np.float32)
         .astype(ml_dtypes.bfloat16))
    col = np.asarray(column_index).astype(np.int64, copy=False)
    dg = np.asarray(degrees, dtype=np.float32)

    # remap node id -> row in the AllGather-concatenated padded table
    col32 = (col // shard * npad + col % shard).astype(np.int32)
    col32 = col32.reshape(n_cores, shard, deg)

    n_tiles = npad // P
    half = deg // 2
    in_maps = []
    pad = npad - shard
    iw = np.arange(half * P)
    for c in range(n_cores):
        dgc = np.concatenate(
            [dg[c * shard:(c + 1) * shard],
             np.zeros(pad, np.float32)], axis=0)
        XcT = np.zeros((X.shape[1], npad), np.float32)
        XcT[:, :shard] = (X[c * shard:(c + 1) * shard] *
                          dg[c * shard:(c + 1) * shard, None]).T
        ixc = np.concatenate(
            [col32[c], np.zeros((pad, deg), np.int32)], axis=0)
        q4, r4 = ixc // 4, ixc % 4                      # [npad, deg]
        # idx16[t, h]: wrapped int16 layout for 1024-idx dma_gather calls;
        # logical i = s_local*128 + p -> wrap[i%16, i//16], tiled to 128 rows
        idx16 = np.zeros((n_tiles, 2, P, half * P // 16), np.int16)
        for t in range(n_tiles):
            blk = q4[t * P:(t + 1) * P]                 # [128, deg]
            for h in range(2):
                arr = blk[:, h * half:(h + 1) * half].T.reshape(-1)
                wrap = np.zeros((16, half * P // 16), np.int16)
                wrap[iw % 16, iw // 16] = arr
                idx16[t, h] = np.tile(wrap, (8, 1))
        # msk[t, p, s*4+q] = d_i where residue matches (d_i folded in)
        msk = (r4[:, :, None] == np.arange(4)[None, None, :]).astype(
            np.float32) * dgc[:, None, None]
        msk = msk.reshape(n_tiles, P, deg * 4).astype(ml_dtypes.bfloat16)
        in_maps.append({"XsT": XcT.astype(ml_dtypes.bfloat16), "W": W,
                        "idxs": idx16, "msks": msk})
    return in_maps


def kernel(X, weights, row_pointers, column_index, degrees):
    from concourse.bass_utils import run_bass_kernel_spmd

    rp = np.asarray(row_pointers)
    assert rp.shape[0] == N_NODES + 1
    in_maps = _prep_inputs(X, weights, column_index, degrees)
    nc = _get_program()
    res = run_bass_kernel_spmd(nc, in_maps, core_ids=list(range(N_CORES)))
    outs = [res.results[c]["out"][:SHARD] for c in range(N_CORES)]
    return np.concatenate(outs, axis=0)


# revision 16
# speedup vs baseline: 2.7848x; 1.1037x over previous
"""GCNConv Trainium2 kernel: out = D^{-1/2} A D^{-1/2} (X @ W).

Strategy (8 NeuronCores, 1D row partition of the uniform-degree CSR):
  - each core owns 12500 destination nodes (padded to 12544 = 98*128)
  - phase A: (d_j X)_k @ W in bf16 (X pre-scaled by d_j and
    pre-transposed on host -> no PE transposes, no post-scale;
    PSUM -> bf16 via the scalar engine), two 128-node tiles per step
  - phase B: AllGather bf16 shards -> full [100352, 64] bf16 table
  - phase C: per 256-node tile-pair, ONE 4096-idx dma_gather
    (round-robin over 4 SWDGE queues) fetches the 16 neighbor rows of
    each node from the 4-packed bf16 table (512B rows, idx = node//4
    fits int16); one fused mask-mult (d_i-premultiplied residue masks)
    + contiguous halving-add tree, store f32.
Host side: shard/pad inputs, fold both degree scalings, transpose X,
remap column indices into the packed AllGather layout.
"""

import numpy as np
import ml_dtypes

N_NODES = 100000
D_IN = 256
D_OUT = 64
DEG = 16
N_CORES = 8
P = 128
SHARD = N_NODES // N_CORES            # 12500
N_TILES = (SHARD + P - 1) // P        # 98
NPAD = N_TILES * P                    # 12544

_CACHE = {}


def _build_program(n_tiles=N_TILES, deg=DEG, d_in=D_IN, d_out=D_OUT,
                   n_cores=N_CORES, debug_taps=False):
    import concourse.bacc as bacc
    from concourse import bass, mybir, tile

    npad = n_tiles * P
    n_pairs = n_tiles // 2
    f32 = mybir.dt.float32
    bf16 = mybir.dt.bfloat16
    i16 = mybir.dt.int16

    nc = bacc.Bacc("TRN2", target_bir_lowering=False, debug=False,
                   num_devices=n_cores, num_swdge_queues=4)
    XsT = nc.dram_tensor("XsT", [d_in, npad], bf16, kind="ExternalInput").ap()
    W = nc.dram_tensor("W", [d_in, d_out], bf16, kind="ExternalInput").ap()
    # packed-gather inputs: node//4 indices (int16, one 4096-idx call per
    # tile-pair, wrapped layout) + residue one-hot masks * d_i
    nidx = 2 * deg * P                     # idxs per pair call
    idxs = nc.dram_tensor("idxs", [n_pairs, P, nidx // 16], i16,
                          kind="ExternalInput").ap()
    msks = nc.dram_tensor("msks", [n_pairs, P, 2 * deg * 4], bf16,
                          kind="ExternalInput").ap()
    out = nc.dram_tensor("out", [npad, d_out], f32, kind="ExternalOutput").ap()
    if debug_taps:
        xpd_out = nc.dram_tensor("xpd_out", [npad, d_out], bf16,
                                 kind="ExternalOutput").ap()
        xfull_out = nc.dram_tensor("xfull_out", [n_cores * npad, d_out], bf16,
                                   kind="ExternalOutput").ap()

    n_kchunk = d_in // P  # 2
    pk = 4 * d_out        # 256 bf16 elems = 512B per packed row
    nm = 2 * deg          # gather slots per partition per pair

    with tile.TileContext(nc) as tc:
        with (
            tc.tile_pool(name="const", bufs=1) as constp,
            tc.tile_pool(name="xin", bufs=3) as xinp,
            tc.tile_pool(name="ps", bufs=4, space="PSUM") as psp,
            tc.tile_pool(name="xp", bufs=3) as xpp,
            tc.tile_pool(name="ix", bufs=4) as idxp,
            tc.tile_pool(name="gt", bufs=3) as gp,
            tc.tile_pool(name="pr", bufs=2) as prp,
            tc.tile_pool(name="rd", bufs=2) as rdp,
            tc.tile_pool(name="ot", bufs=3) as outp,
            tc.tile_pool(name="dram", bufs=1, space="DRAM") as dramp,
        ):
            w_sb = constp.tile([P, n_kchunk * d_out], bf16)
            for c in range(n_kchunk):
                nc.sync.dma_start(out=w_sb[:, c * d_out:(c + 1) * d_out],
                                  in_=W[c * P:(c + 1) * P, :])

            xpd = dramp.tile([npad, d_out], bf16)
            xfull = dramp.tile([n_cores * npad, d_out], bf16,
                               addr_space="Shared")

            # ---- Phase A: X'' = (d_j X) @ W  (bf16), 2 tiles per step ----
            for r in range(n_pairs):
                sl2 = slice(r * 2 * P, (r + 1) * 2 * P)
                xT = xinp.tile([P, n_kchunk, 2 * P], bf16)
                for c in range(n_kchunk):
                    nc.sync.dma_start(out=xT[:, c, :],
                                      in_=XsT[c * P:(c + 1) * P, sl2])
                xp_t = xpp.tile([P, 2, d_out], bf16)
                for j in range(2):
                    pso = psp.tile([P, d_out], f32, space="PSUM", tag=f"ps{j}")
                    for c in range(n_kchunk):
                        nc.tensor.matmul(
                            pso[:], xT[:, c, j * P:(j + 1) * P],
                            w_sb[:, c * d_out:(c + 1) * d_out],
                            start=(c == 0), stop=(c == n_kchunk - 1))
                    nc.scalar.copy(xp_t[:, j, :], pso[:])
                nc.sync.dma_start(
                    out=xpd[sl2, :].rearrange("(j p) f -> p j f", p=P),
                    in_=xp_t[:])

            # ---- Phase B: AllGather shards ----
            nc.gpsimd.collective_compute(
                "AllGather", mybir.AluOpType.bypass,
                replica_groups=[list(range(n_cores))],
                ins=[xpd.opt()], outs=[xfull.opt()],
            )

            if debug_taps:
                nc.sync.dma_start(out=xpd_out[:], in_=xpd[:])
                nc.sync.dma_start(out=xfull_out[:], in_=xfull[:])

            # ---- Phase C: one 4096-idx dma_gather per tile-pair on the
            # 4-packed bf16 table; fused mask-mult + halving-add tree ----
            xpk = xfull[:].rearrange("(a b) f -> a (b f)", b=4)  # [N/4, 256]
            for r in range(n_pairs):
                sl2 = slice(r * 2 * P, (r + 1) * 2 * P)
                msk_t = idxp.tile([P, nm * 4], bf16, tag="msk")
                nc.sync.dma_start(out=msk_t[:], in_=msks[r])
                idx_t = idxp.tile([P, nidx // 16], i16, tag="idx")
                nc.sync.dma_start(out=idx_t[:], in_=idxs[r])
                g = gp.tile([P, nm * pk], bf16, tag="g")
                nsp = 4  # gather calls per pair (1024 idxs each)
                for h in range(nsp):
                    hm = nm // nsp
                    nc.gpsimd.dma_gather(
                        g[:, h * hm * pk:(h + 1) * hm * pk]
                        .rearrange("p (m f) -> p m f", m=hm),
                        xpk,
                        idx_t[:, h * (nidx // (16 * nsp)):
                              (h + 1) * (nidx // (16 * nsp))],
                        nidx // nsp, nidx // nsp, pk,
                        queue_num=(nsp * r + h) % 4)
                # prod[p, (m q), f] = g * msk  (mask has d_i folded in)
                prod = prp.tile([P, nm * pk], bf16, tag="prod")
                nc.vector.tensor_tensor(
                    out=prod[:].rearrange("p (mq f) -> p mq f", f=d_out),
                    in0=g[:].rearrange("p (mq f) -> p mq f", f=d_out),
                    in1=msk_t[:].rearrange("p (mq o) -> p mq o", o=1)
                    .to_broadcast([P, nm * 4, d_out]),
                    op=mybir.AluOpType.mult)
                # halving-add tree over s (contiguous), then q (2-run views)
                width = nm * pk // 2           # after first s-halve
                cur = prod
                step = 0
                while width >= 2 * 4 * d_out:  # down to (j, q, f)
                    nxt = rdp.tile([P, width], bf16, tag=f"h{step}")
                    nc.vector.tensor_add(nxt[:], cur[:, :width],
                                         cur[:, width:2 * width])
                    cur, step, width = nxt, step + 1, width // 2
                q4v = cur[:].rearrange("p (j q f) -> p j q f", j=2, q=4)
                qh = rdp.tile([P, 2 * 2 * d_out], bf16, tag="qh")
                nc.vector.tensor_add(
                    qh[:].rearrange("p (j q f) -> p j q f", j=2, q=2),
                    q4v[:, :, 0:2, :], q4v[:, :, 2:4, :])
                qhv = qh[:].rearrange("p (j q f) -> p j q f", j=2, q=2)
                o_t = outp.tile([P, 2, d_out], f32, tag="o_t")
                nc.vector.tensor_add(o_t[:], qhv[:, :, 0, :], qhv[:, :, 1, :])
                nc.sync.dma_start(
                    out=out[sl2, :].rearrange("(j p) f -> p j f", p=P),
                    in_=o_t[:])

    nc.compile()
    return nc


def _get_program():
    key = "main"
    if key not in _CACHE:
        _CACHE[key] = _build_program()
    return _CACHE[key]


def _prep_inputs(X, weights, column_index, degrees,
                 n_nodes=N_NODES, n_cores=N_CORES, shard=SHARD, npad=NPAD,
                 deg=DEG):
    """Shard + pad host arrays; fold degree scalings; remap columns to the
    packed AllGather layout (pair-merged gather order)."""
    X = np.asarray(X, dtype=np.float32)
    W = (np.asarray(weights, dtype=np.float32)
         .astype(ml_dtypes.bfloat16))
    col = np.asarray(column_index).astype(np.int64, copy=False)
    dg = np.asarray(degrees, dtype=np.float32)

    # remap node id -> row in the AllGather-concatenated padded table
    col32 = (col // shard * npad + col % shard).astype(np.int32)
    col32 = col32.reshape(n_cores, shard, deg)

    n_tiles = npad // P
    n_pairs = n_tiles // 2
    nidx = 2 * deg * P
    in_maps = []
    pad = npad - shard
    iw = np.arange(nidx)
    for c in range(n_cores):
        dgc = np.concatenate(
            [dg[c * shard:(c + 1) * shard],
             np.zeros(pad, np.float32)], axis=0)
        XcT = np.zeros((X.shape[1], npad), np.float32)
        XcT[:, :shard] = (X[c * shard:(c + 1) * shard] *
                          dg[c * shard:(c + 1) * shard, None]).T
        ixc = np.concatenate(
            [col32[c], np.zeros((pad, deg), np.int32)], axis=0)
        q4, r4 = ixc // 4, ixc % 4                      # [npad, deg]
        # gather position i = (s*2 + j)*128 + p  (slot-major, pair-interleaved)
        # wrapped: wrap[i%16, i//16], tiled to 128 partitions
        q4p = q4.reshape(n_pairs, 2, P, deg)            # [r, j, p, s]
        arr = q4p.transpose(0, 3, 1, 2).reshape(n_pairs, nidx)  # (s, j, p)
        idx16 = np.zeros((n_pairs, P, nidx // 16), np.int16)
        wrap = np.zeros((16, nidx // 16), np.int16)
        for r in range(n_pairs):
            wrap[iw % 16, iw // 16] = arr[r]
            idx16[r] = np.tile(wrap, (8, 1))
        # msk[r, p, ((s j) q)] = d_i(r, j, p) where residue matches
        oh = (r4[:, :, None] == np.arange(4)[None, None, :]).astype(
            np.float32) * dgc[:, None, None]            # [npad, s, q]
        ohp = oh.reshape(n_pairs, 2, P, deg, 4)         # [r, j, p, s, q]
        msk = (ohp.transpose(0, 2, 3, 1, 4)             # [r, p, s, j, q]
               .reshape(n_pairs, P, 2 * deg * 4)
               .astype(ml_dtypes.bfloat16))
        in_maps.append({"XsT": XcT.astype(ml_dtypes.bfloat16), "W": W,
                        "idxs": idx16, "msks": msk})
    return in_maps


def kernel(X, weights, row_pointers, column_index, degrees):
    from concourse.bass_utils import run_bass_kernel_spmd

    rp = np.asarray(row_pointers)
    assert rp.shape[0] == N_NODES + 1
    in_maps = _prep_inputs(X, weights, column_index, degrees)
    nc = _get_program()
    res = run_bass_kernel_spmd(nc, in_maps, core_ids=list(range(N_CORES)))
    outs = [res.results[c]["out"][:SHARD] for c in range(N_CORES)]
    return np.concatenate(outs, axis=0)


# revision 19
# speedup vs baseline: 3.8633x; 1.3873x over previous
"""GCNConv Trainium2 kernel: out = D^{-1/2} A D^{-1/2} (X @ W).

Strategy (8 NeuronCores, 1D row partition of the uniform-degree CSR):
  - each core owns 12500 destination nodes (padded to 12544 = 98*128)
  - phase A: (d_j X)_k @ W in bf16 (X pre-scaled by d_j and
    pre-transposed on host -> no PE transposes, no post-scale;
    PSUM -> bf16 via the scalar engine), two 128-node tiles per step
  - phase B: AllGather bf16 shards -> full [100352, 64] bf16 table
  - phase C: per 256-node tile-pair, ONE 4096-idx dma_gather
    (round-robin over 4 SWDGE queues) fetches the 16 neighbor rows of
    each node from the 4-packed bf16 table (512B rows, idx = node//4
    fits int16); one fused mask-mult (d_i-premultiplied residue masks)
    + contiguous halving-add tree, store f32.
Host side: shard/pad inputs, fold both degree scalings, transpose X,
remap column indices into the packed AllGather layout.
"""

import numpy as np
import ml_dtypes

N_NODES = 100000
D_IN = 256
D_OUT = 64
DEG = 16
N_CORES = 8
P = 128
SHARD = N_NODES // N_CORES            # 12500
N_TILES = (SHARD + P - 1) // P        # 98
NPAD = N_TILES * P                    # 12544

_CACHE = {}


def _build_program(n_tiles=N_TILES, deg=DEG, d_in=D_IN, d_out=D_OUT,
                   n_cores=N_CORES, debug_taps=False):
    import concourse.bacc as bacc
    from concourse import bass, mybir, tile

    npad = n_tiles * P
    n_pairs = n_tiles // 2
    f32 = mybir.dt.float32
    bf16 = mybir.dt.bfloat16
    i16 = mybir.dt.int16

    nc = bacc.Bacc("TRN2", target_bir_lowering=False, debug=False,
                   num_devices=n_cores, num_swdge_queues=4)
    XsT = nc.dram_tensor("XsT", [d_in, npad], bf16, kind="ExternalInput").ap()
    W = nc.dram_tensor("W", [d_in, d_out], bf16, kind="ExternalInput").ap()
    # packed-gather inputs: node//4 indices (int16, one 4096-idx call per
    # tile-pair, wrapped layout) + residue one-hot masks * d_i
    nidx = 2 * deg * P                     # idxs per pair call
    idxs = nc.dram_tensor("idxs", [n_pairs, P, nidx // 16], i16,
                          kind="ExternalInput").ap()
    msks = nc.dram_tensor("msks", [n_pairs, P, 2 * deg * 4],
                          mybir.dt.int32, kind="ExternalInput").ap()
    degs = nc.dram_tensor("degs", [npad, 1], f32, kind="ExternalInput").ap()
    out = nc.dram_tensor("out", [npad, d_out], f32, kind="ExternalOutput").ap()
    if debug_taps:
        xpd_out = nc.dram_tensor("xpd_out", [npad, d_out], bf16,
                                 kind="ExternalOutput").ap()
        xfull_out = nc.dram_tensor("xfull_out", [n_cores * npad, d_out], bf16,
                                   kind="ExternalOutput").ap()

    n_kchunk = d_in // P  # 2
    pk = 4 * d_out        # 256 bf16 elems = 512B per packed row
    nm = 2 * deg          # gather slots per partition per pair

    with tile.TileContext(nc) as tc:
        with (
            tc.tile_pool(name="const", bufs=1) as constp,
            tc.tile_pool(name="xin", bufs=3) as xinp,
            tc.tile_pool(name="ps", bufs=2, space="PSUM") as psp,
            tc.tile_pool(name="xp", bufs=3) as xpp,
            tc.tile_pool(name="ix", bufs=4) as idxp,
            tc.tile_pool(name="gt", bufs=3) as gp,
            tc.tile_pool(name="pr", bufs=2) as prp,
            tc.tile_pool(name="rd", bufs=2) as rdp,
            tc.tile_pool(name="ot", bufs=3) as outp,
            tc.tile_pool(name="dram", bufs=1, space="DRAM") as dramp,
        ):
            w_sb = constp.tile([P, n_kchunk * d_out], bf16)
            for c in range(n_kchunk):
                nc.sync.dma_start(out=w_sb[:, c * d_out:(c + 1) * d_out],
                                  in_=W[c * P:(c + 1) * P, :])

            xpd = dramp.tile([npad, d_out], bf16)
            xfull = dramp.tile([n_cores * npad, d_out], bf16,
                               addr_space="Shared")

            # ---- Phase A: X'' = (d_j X) @ W  (bf16), 4 tiles per step ----
            ga_groups = [4] * (n_tiles // 4) + ([2] if n_tiles % 4 else [])
            ga_off = 0
            for gs in ga_groups:
                slg = slice(ga_off * P, (ga_off + gs) * P)
                xT = xinp.tile([P, n_kchunk, 4 * P], bf16, tag="xT")
                for c in range(n_kchunk):
                    nc.sync.dma_start(out=xT[:, c, :gs * P],
                                      in_=XsT[c * P:(c + 1) * P, slg])
                xp_t = xpp.tile([P, 4, d_out], bf16, tag="xp")
                for j in range(gs):
                    pso = psp.tile([P, d_out], f32, space="PSUM", tag=f"ps{j}")
                    for c in range(n_kchunk):
                        nc.tensor.matmul(
                            pso[:], xT[:, c, j * P:(j + 1) * P],
                            w_sb[:, c * d_out:(c + 1) * d_out],
                            start=(c == 0), stop=(c == n_kchunk - 1))
                    nc.scalar.copy(xp_t[:, j, :], pso[:])
                nc.sync.dma_start(
                    out=xpd[slg, :].rearrange("(j p) f -> p j f", p=P),
                    in_=xp_t[:, :gs, :])
                ga_off += gs

            # ---- Phase B: AllGather shards ----
            nc.gpsimd.collective_compute(
                "AllGather", mybir.AluOpType.bypass,
                replica_groups=[list(range(n_cores))],
                ins=[xpd.opt()], outs=[xfull.opt()],
            )

            if debug_taps:
                nc.sync.dma_start(out=xpd_out[:], in_=xpd[:])
                nc.sync.dma_start(out=xfull_out[:], in_=xfull[:])

            # ---- Phase C: one 4096-idx dma_gather per tile-pair on the
            # 4-packed bf16 table; fused mask-mult + halving-add tree ----
            xpk = xfull[:].rearrange("(a b) f -> a (b f)", b=4)  # [N/4, 256]
            for r in range(n_pairs):
                sl2 = slice(r * 2 * P, (r + 1) * 2 * P)
                msk_t = idxp.tile([P, nm * 4], mybir.dt.int32, tag="msk")
                nc.sync.dma_start(out=msk_t[:], in_=msks[r])
                deg_c = idxp.tile([P, 2], f32, tag="deg")
                nc.sync.dma_start(
                    out=deg_c[:],
                    in_=degs[sl2, :].rearrange("(j p) o -> p (j o)", p=P))
                idx_t = idxp.tile([P, nidx // 16], i16, tag="idx")
                nc.sync.dma_start(out=idx_t[:], in_=idxs[r])
                g = gp.tile([P, nm * pk], bf16, tag="g")
                nsp = 4  # gather calls per pair (1024 idxs each)
                for h in range(nsp):
                    hm = nm // nsp
                    nc.gpsimd.dma_gather(
                        g[:, h * hm * pk:(h + 1) * hm * pk]
                        .rearrange("p (m f) -> p m f", m=hm),
                        xpk,
                        idx_t[:, h * (nidx // (16 * nsp)):
                              (h + 1) * (nidx // (16 * nsp))],
                        nidx // nsp, nidx // nsp, pk,
                        queue_num=(nsp * r + h) % 4)
                # prod = g & msk: residue select as int32 bitwise AND
                # (halves DVE elems vs a bf16 broadcast multiply)
                prod = prp.tile([P, nm * pk], bf16, tag="prod")
                i32f = d_out // 2
                nc.vector.tensor_tensor(
                    out=prod[:].bitcast(mybir.dt.int32)
                    .rearrange("p (mq f) -> p mq f", f=i32f),
                    in0=g[:].bitcast(mybir.dt.int32)
                    .rearrange("p (mq f) -> p mq f", f=i32f),
                    in1=msk_t[:].rearrange("p (mq o) -> p mq o", o=1)
                    .to_broadcast([P, nm * 4, i32f]),
                    op=mybir.AluOpType.bitwise_and)
                # halving-add tree over s (contiguous), then q (2-run views)
                width = nm * pk // 2           # after first s-halve
                cur = prod
                step = 0
                while width >= 2 * 4 * d_out:  # down to (j, q, f)
                    nxt = rdp.tile([P, width], bf16, tag=f"h{step}")
                    nc.vector.tensor_add(nxt[:], cur[:, :width],
                                         cur[:, width:2 * width])
                    cur, step, width = nxt, step + 1, width // 2
                q4v = cur[:].rearrange("p (j q f) -> p j q f", j=2, q=4)
                qh = rdp.tile([P, 2 * 2 * d_out], bf16, tag="qh")
                nc.vector.tensor_add(
                    qh[:].rearrange("p (j q f) -> p j q f", j=2, q=2),
                    q4v[:, :, 0:2, :], q4v[:, :, 2:4, :])
                qhv = qh[:].rearrange("p (j q f) -> p j q f", j=2, q=2)
                rsum = outp.tile([P, 2, d_out], f32, tag="rsum")
                nc.vector.tensor_add(rsum[:], qhv[:, :, 0, :],
                                     qhv[:, :, 1, :])
                o_t = outp.tile([P, 2, d_out], f32, tag="o_t")
                nc.vector.tensor_tensor(
                    out=o_t[:], in0=rsum[:],
                    in1=deg_c[:].rearrange("p (j o) -> p j o", o=1)
                    .to_broadcast([P, 2, d_out]),
                    op=mybir.AluOpType.mult)
                nc.sync.dma_start(
                    out=out[sl2, :].rearrange("(j p) f -> p j f", p=P),
                    in_=o_t[:])

    nc.compile()
    return nc


def _get_program():
    key = "main"
    if key not in _CACHE:
        _CACHE[key] = _build_program()
    return _CACHE[key]


def _prep_inputs(X, weights, column_index, degrees,
                 n_nodes=N_NODES, n_cores=N_CORES, shard=SHARD, npad=NPAD,
                 deg=DEG):
    """Shard + pad host arrays; fold degree scalings; remap columns to the
    packed AllGather layout (pair-merged gather order)."""
    X = np.asarray(X, dtype=np.float32)
    W = (np.asarray(weights, dtype=np.float32)
         .astype(ml_dtypes.bfloat16))
    col = np.asarray(column_index).astype(np.int64, copy=False)
    dg = np.asarray(degrees, dtype=np.float32)

    # remap node id -> row in the AllGather-concatenated padded table
    col32 = (col // shard * npad + col % shard).astype(np.int32)
    col32 = col32.reshape(n_cores, shard, deg)

    n_tiles = npad // P
    n_pairs = n_tiles // 2
    nidx = 2 * deg * P
    in_maps = []
    pad = npad - shard
    iw = np.arange(nidx)
    for c in range(n_cores):
        dgc = np.concatenate(
            [dg[c * shard:(c + 1) * shard],
             np.zeros(pad, np.float32)], axis=0)
        XcT = np.zeros((X.shape[1], npad), np.float32)
        XcT[:, :shard] = (X[c * shard:(c + 1) * shard] *
                          dg[c * shard:(c + 1) * shard, None]).T
        ixc = np.concatenate(
            [col32[c], np.zeros((pad, deg), np.int32)], axis=0)
        q4, r4 = ixc // 4, ixc % 4                      # [npad, deg]
        # gather position i = (s*2 + j)*128 + p  (slot-major, pair-interleaved)
        # wrapped: wrap[i%16, i//16], tiled to 128 partitions
        q4p = q4.reshape(n_pairs, 2, P, deg)            # [r, j, p, s]
        arr = q4p.transpose(0, 3, 1, 2).reshape(n_pairs, nidx)  # (s, j, p)
        idx16 = np.zeros((n_pairs, P, nidx // 16), np.int16)
        wrap = np.zeros((16, nidx // 16), np.int16)
        for r in range(n_pairs):
            wrap[iw % 16, iw // 16] = arr[r]
            idx16[r] = np.tile(wrap, (8, 1))
        # msk[r, p, ((s j) q)] = -1 (all bits) where residue matches
        oh = np.where(r4[:, :, None] == np.arange(4)[None, None, :],
                      np.int32(-1), np.int32(0))        # [npad, s, q]
        ohp = oh.reshape(n_pairs, 2, P, deg, 4)         # [r, j, p, s, q]
        msk = (ohp.transpose(0, 2, 3, 1, 4)             # [r, p, s, j, q]
               .reshape(n_pairs, P, 2 * deg * 4).copy())
        in_maps.append({"XsT": XcT.astype(ml_dtypes.bfloat16), "W": W,
                        "idxs": idx16, "msks": msk,
                        "degs": dgc.reshape(npad, 1)})
    return in_maps


def kernel(X, weights, row_pointers, column_index, degrees):
    from concourse.bass_utils import run_bass_kernel_spmd

    rp = np.asarray(row_pointers)
    assert rp.shape[0] == N_NODES + 1
    in_maps = _prep_inputs(X, weights, column_index, degrees)
    nc = _get_program()
    res = run_bass_kernel_spmd(nc, in_maps, core_ids=list(range(N_CORES)))
    outs = [res.results[c]["out"][:SHARD] for c in range(N_CORES)]
    return np.concatenate(outs, axis=0)
